# revision 16
# baseline (speedup 1.0000x reference)
"""AFM (attentional factorization machine) forward on 8 TRN2 NeuronCores.

Math: the reference's attention branch is dead — softmax over a size-1 axis
is identically 1.0, so

    out[b] = sigmoid(out_kernel * sum_{i<j,k} e_i[k] e_j[k] + out_bias)
           = sigmoid(out_kernel * 0.5 * (||sum_f e_f||^2 - sum_f ||e_f||^2) + out_bias)

where e_f = embed_tables[f, sparse_ids[b, f], :].  The kernel is therefore an
embedding gather (memory-bound) plus tiny reductions.

Sharding: data-parallel over batch; each of the 8 cores handles 512 rows and
holds a full replica of the embedding tables (params staged to device DRAM
before NEFF execution).  No collectives needed; host concatenates the shards.

Implementation: raw Bass (no Tile) — 4 engines, 4 semaphores:
  SP:   input DMA -> ... -> output DMA
  DVE:  idx = ids + f*VOCAB; S = sum_f e; SS = ||S||^2; x = SS - Q
  Pool: indirect-DMA gather of 13312 rows x 64B
  ACT:  Q via Square+accum_out (4 slices); y = sigmoid(x*0.5w + b)
"""

import numpy as np

import concourse.bass as bass
import concourse.mybir as mybir
from concourse.bass_utils import run_bass_kernel_spmd

N_CORES = 8
B = 4096
NF = 26          # sparse fields
EMB = 16
VOCAB = 100000
P = 128          # SBUF partitions
B_SHARD = B // N_CORES          # 512 rows per core
QQ = B_SHARD // P               # 4 rows per partition  (b = p*QQ + q)
FK = NF * EMB                   # 416 floats per row
COLS = QQ * FK                  # 1664 floats per partition
IDC = QQ * NF                   # 104 index columns
HDR = 2 * IDC + 2               # 210: [ids | f*VOCAB | w.bits | b.bits]

F32 = mybir.dt.float32
I32 = mybir.dt.int32
AF = mybir.ActivationFunctionType


def build_nc() -> bass.Bass:
    nc = bass.Bass()

    hdr_ext = nc.declare_dram_parameter("hdr", [P, HDR], I32, isOutput=False)
    tab_ext = nc.declare_dram_parameter("embed_tables", [NF * VOCAB, EMB], F32, isOutput=False)
    out_ext = nc.declare_dram_parameter("out", [B_SHARD, 1], F32, isOutput=True)

    with (
        nc.sbuf_tensor([P, HDR], I32) as hdr,
        nc.sbuf_tensor([P, IDC], I32) as idx,
        nc.sbuf_tensor([P, COLS], F32) as g,
        nc.sbuf_tensor([P, COLS], F32) as sqs,
        nc.sbuf_tensor([P, QQ * EMB], F32) as s,
        nc.sbuf_tensor([P, QQ * EMB], F32) as s2,
        nc.sbuf_tensor([P, QQ], F32) as ss,
        nc.sbuf_tensor([P, QQ], F32) as qv,
        nc.sbuf_tensor([P, QQ], F32) as x,
        nc.sbuf_tensor([P, QQ], F32) as y,
        nc.sbuf_tensor([P, 1], F32) as wh,
        nc.semaphore("d_sem") as d_sem,
        nc.semaphore("v_sem") as v_sem,
        nc.semaphore("gq0") as gq0,
        nc.semaphore("gq1") as gq1,
        nc.semaphore("gq2") as gq2,
        nc.semaphore("gq3") as gq3,
        nc.semaphore("a_sem") as a_sem,
        nc.Block() as block,
    ):
        gq = [gq0, gq1, gq2, gq3]
        w_ap = hdr[:, 2 * IDC : 2 * IDC + 1].bitcast(F32)
        b_ap = hdr[:, 2 * IDC + 1 : 2 * IDC + 2].bitcast(F32)

        @block.sync
        def _(sync):
            sync.dma_start(out=hdr[:], in_=hdr_ext[:]).then_inc(d_sem, 16)
            sync.wait_ge(a_sem, QQ + 1)
            sync.dma_start(
                out=out_ext.rearrange("(p q) o -> p (q o)", p=P), in_=y[:]
            ).then_inc(d_sem, 16)
            sync.wait_ge(d_sem, 32)

        @block.vector
        def _(vector):
            vector.wait_ge(d_sem, 16)
            vector.tensor_add(idx[:], hdr[:, :IDC], hdr[:, IDC : 2 * IDC]).then_inc(
                v_sem, 1
            )
            # S[p, q, k] = sum_f g[p, q, f, k] -- one strided reduce per q,
            # pipelined against the gather stream (columns arrive in order)
            for q in range(QQ):
                vector.wait_ge(gq[q], 16 * NF)
                vector.reduce_sum(
                    s[:, q * EMB : (q + 1) * EMB],
                    g[:, q * FK : (q + 1) * FK].rearrange(
                        "p (f k) -> p k f", f=NF, k=EMB
                    ),
                    axis=mybir.AxisListType.X,
                ).then_inc(v_sem, 1)
            vector.wait_ge(v_sem, 1 + QQ)
            vector.tensor_mul(s2[:], s[:], s[:]).then_inc(v_sem, 1)
            vector.wait_ge(v_sem, 2 + QQ)
            vector.reduce_sum(
                ss[:],
                s2[:].rearrange("p (q k) -> p q k", q=QQ),
                axis=mybir.AxisListType.X,
            ).then_inc(v_sem, 1)
            vector.tensor_scalar_mul(wh[:], w_ap, 0.5).then_inc(v_sem, 1)
            vector.wait_ge(v_sem, 3 + QQ)
            vector.wait_ge(a_sem, QQ)
            vector.tensor_tensor(
                x[:], ss[:], qv[:], op=mybir.AluOpType.subtract
            ).then_inc(v_sem, 1)  # v = 5 + QQ

        @block.gpsimd
        def _(gpsimd):
            gpsimd.wait_ge(v_sem, 1)
            # canonical indirect gathers: one instr per column c -- the only
            # HW-supported form (one offset per partition, dest [P, EMB])
            for c in range(IDC):
                gpsimd.indirect_dma_start(
                    out=g[:, c * EMB : (c + 1) * EMB],
                    out_offset=None,
                    in_=tab_ext[:],
                    in_offset=bass.IndirectOffsetOnAxis(ap=idx[:, c : c + 1], axis=0),
                ).then_inc(gq[c // NF], 16)

        @block.scalar
        def _(scalar):
            scalar.wait_ge(d_sem, 16)
            for q in range(QQ):
                scalar.wait_ge(gq[q], 16 * NF)
                scalar.activation(
                    sqs[:, q * FK : (q + 1) * FK],
                    g[:, q * FK : (q + 1) * FK],
                    AF.Square,
                    accum_out=qv[:, q : q + 1],
                ).then_inc(a_sem, 1)
            scalar.wait_ge(v_sem, 5 + QQ)
            scalar.activation(
                y[:], x[:], AF.Sigmoid, bias=b_ap, scale=wh[:]
            ).then_inc(a_sem, 1)

    return nc


_NC_CACHE = None


def _get_nc() -> bass.Bass:
    global _NC_CACHE
    if _NC_CACHE is None:
        _NC_CACHE = build_nc()
    return _NC_CACHE


def make_hdr(ids_shard: np.ndarray, w: np.float32, bb: np.float32) -> np.ndarray:
    """[128, 210] int32: [ids (q f) | f*VOCAB offsets | w bits | b bits]."""
    foffs = np.tile(
        ((np.arange(IDC, dtype=np.int64) % NF) * VOCAB).astype(np.int32), (P, 1)
    )
    wb_bits = np.tile(
        np.array([[w, bb]], dtype=np.float32).view(np.int32), (P, 1)
    )
    return np.ascontiguousarray(
        np.concatenate([ids_shard.reshape(P, IDC), foffs, wb_bits], axis=1)
    )


def make_in_maps(inputs: dict) -> list[dict]:
    ids = np.ascontiguousarray(np.asarray(inputs["sparse_ids"], dtype=np.int32))
    tab = np.ascontiguousarray(np.asarray(inputs["embed_tables"], dtype=np.float32))
    tab = tab.reshape(NF * VOCAB, EMB)
    w = np.float32(np.asarray(inputs["out_kernel"]).reshape(()))
    bb = np.float32(np.asarray(inputs["out_bias"]).reshape(()))
    in_maps = []
    for c in range(N_CORES):
        in_maps.append(
            {
                "hdr": make_hdr(ids[c * B_SHARD : (c + 1) * B_SHARD], w, bb),
                "embed_tables": tab,
            }
        )
    return in_maps


def run(inputs: dict, **spmd_kwargs):
    """Run on hardware; returns (full_output [4096,1] f32, BassKernelResults)."""
    nc = _get_nc()
    in_maps = make_in_maps(inputs)
    res = run_bass_kernel_spmd(nc, in_maps, core_ids=list(range(N_CORES)), **spmd_kwargs)
    outs = [
        np.asarray(res.results[i]["out"], dtype=np.float32).reshape(B_SHARD, 1)
        for i in range(N_CORES)
    ]
    return np.concatenate(outs, axis=0), res


def kernel(**inputs) -> np.ndarray:
    out, _ = run(inputs)
    return out
